# revision 18
# baseline (speedup 1.0000x reference)
"""Trainium2 Bass kernel for nn_CliffordJEPAModel.

Model = two GRU encoders (ctx / tgt) + tiny closed-form head.

Sharding: 8 cores = 2 encoders x 4 batch-quarters (B_local=16), no
cross-core communication.

Per-core program:
  phase 1+2: gather X^T chunks (fp16) and compute gi^T = Wih' @ X^T + bias,
             stored m-major to a DRAM scratch (fp16).
  phase 3:   256 sequential GRU steps.  The 18 m-tiles (gate rows) are
             organized into NG=3 groups of 2 dim-chunks; each group gets
             its own PSUM bank so its gate chain can run while the PE
             accumulates the other groups.  MM burst is k-outer so the
             next step consumes h chunks in the order the groups finish.
             Gate math: h' = (1-z)*n + z*h with
               u = (gh_n + bhh_n) * r       (fused scalar_tensor_tensor)
               v = u + gi_n  -> n = tanh(v)
               q = 1 - z ; zh = z*h ; h' = q*n + zh   (h kept in fp16)
  output:    final h^T  [128, 6*16] fp32.

Host does the final tiny head math in numpy (fc -> spectral norm ->
closed-form descent).
"""

import sys

for _p in ("/opt/trn_rl_repo/concourse", "/opt/trn_rl_repo"):
    if _p not in sys.path:
        sys.path.insert(0, _p)

import numpy as np
import ml_dtypes

import concourse.bacc as bacc
import concourse.mybir as mybir
import concourse.tile as tile
from concourse.bass_utils import run_bass_kernel_spmd

FP16 = ml_dtypes.float16 if hasattr(ml_dtypes, "float16") else np.float16

V, D, NB = 32000, 768, 8
B, S = 64, 256
DT_STEP, STEPS_DESC, PI = 0.1, 5, 3

N_CORES = 8
BQ = B // 4          # batch rows per core (16)
KT = D // 128        # 6 k-chunks
NG = 3               # gate groups per step
CPG = KT // NG       # dim-chunks per group (2)
MT = 3 * KT          # 18 m-tiles of 128 gate rows
NT = BQ * S          # tokens per core (4096)
CHT = 512            # tokens per gather/input-matmul chunk
NCH = NT // CHT      # 8 chunks
BLK = 16             # recurrence steps per gi prefetch block

F32 = mybir.dt.float32
F16 = mybir.dt.float16
I16 = mybir.dt.int16
AF = mybir.ActivationFunctionType
ALU = mybir.AluOpType

# m-tile order: group G covers dim-chunks (2G, 2G+1); within a group the
# m order is [r_c0, r_c1, z_c0, z_c1, n_c0, n_c1] so the r block, z block
# and n block of the psum/gi image are each contiguous. m_list[m] = (c, g).
m_list = []
for G in range(NG):
    for g in range(3):
        for cl in range(CPG):
            m_list.append((2 * G + cl, g))

# gate-row permutation for packing Wih/Whh rows in m_list order
_PERM = np.concatenate(
    [np.arange(g * D + c * 128, g * D + (c + 1) * 128) for (c, g) in m_list]
)


def _build_program(steps=S):
    nc = bacc.Bacc("TRN2", target_bir_lowering=False, debug=False, num_devices=N_CORES)

    t_idx = nc.dram_tensor("idx", [128, NT // 16], I16, kind="ExternalInput")
    t_emb = nc.dram_tensor("emb", [V, D], F16, kind="ExternalInput")
    t_wih = nc.dram_tensor("wihT", [128, KT * 3 * D], F16, kind="ExternalInput")
    t_whh = nc.dram_tensor("whhT", [128, KT * 3 * D], F16, kind="ExternalInput")
    t_bi = nc.dram_tensor("bias_i", [128, MT], F32, kind="ExternalInput")
    t_bn = nc.dram_tensor("bhhn", [128, KT], F32, kind="ExternalInput")
    t_out = nc.dram_tensor("h_out", [128, KT * BQ], F32, kind="ExternalOutput")

    W3D = 3 * D  # 2304

    with tile.TileContext(nc) as tc:
        with (
            tc.tile_pool(name="const", bufs=1) as const_pool,
            tc.tile_pool(name="dram", bufs=1, space="DRAM") as dram_pool,
        ):
            idx_t = const_pool.tile([128, NT // 16], I16)
            wih_t = const_pool.tile([128, KT * W3D], F16)
            whh_t = const_pool.tile([128, KT * W3D], F16)
            bi_t = const_pool.tile([128, MT], F32)
            bn_t = const_pool.tile([128, KT], F32)
            nc.sync.dma_start(idx_t[:], t_idx.ap())
            nc.sync.dma_start(wih_t[:], t_wih.ap())
            nc.sync.dma_start(whh_t[:], t_whh.ap())
            nc.sync.dma_start(bi_t[:], t_bi.ap())
            nc.sync.dma_start(bn_t[:], t_bn.ap())

            # gi scratch, m-major: giD[m][p][token]  (token = t*16 + b)
            giD = dram_pool.tile([MT, 128, NT], F16)

            # ---- phase 1+2: gather + input matmul -> giD ----
            with (
                tc.tile_pool(name="xt", bufs=3) as xt_pool,
                tc.tile_pool(name="psum_in", bufs=4, space="PSUM") as psum_in,
                tc.tile_pool(name="gis", bufs=4) as gis_pool,
            ):
                for nch in range(NCH):
                    xt = xt_pool.tile([128, KT, CHT], F16)
                    nc.gpsimd.dma_gather(
                        xt[:, :, :],
                        t_emb.ap(),
                        idx_t[:, nch * (CHT // 16):(nch + 1) * (CHT // 16)],
                        num_idxs=CHT,
                        num_idxs_reg=CHT,
                        elem_size=D,
                        transpose=True,
                    )
                    for m in range(MT):
                        ps = psum_in.tile([128, CHT], F32)
                        for k in range(KT):
                            nc.tensor.matmul(
                                ps[:],
                                wih_t[:, k * W3D + m * 128:k * W3D + (m + 1) * 128],
                                xt[:, k, :],
                                start=(k == 0),
                                stop=(k == KT - 1),
                            )
                        gs = gis_pool.tile([128, CHT], F16)
                        nc.scalar.activation(gs[:], ps[:], AF.Identity, bias=bi_t[:, m:m + 1], scale=1.0)
                        nc.sync.dma_start(giD[m, :, nch * CHT:(nch + 1) * CHT], gs[:])

            # ---- phase 3: recurrence ----
            with (
                tc.tile_pool(name="gh", bufs=2 * NG, space="PSUM") as gh_pool,
                tc.tile_pool(name="giblk", bufs=2) as giblk_pool,
                tc.tile_pool(name="hstate", bufs=1) as h_pool,
                tc.tile_pool(name="tmp", bufs=3) as tmp,
            ):
                # h state, fp16, chunk-major [128, k, b]
                h_st = [h_pool.tile([128, KT, BQ], F16, name=f"h{i}", tag=f"h{i}") for i in range(2)]
                import os as _os3
                nc.vector.memset(h_st[0][:], float(_os3.environ.get("H0CONST", "0.0")))

                blk_sz = min(BLK, steps)
                nblk = steps // blk_sz
                GW = CPG * 3 * BQ        # psum cols per group (96)
                RZW = CPG * 2 * BQ       # rz cols per group (64)

                # deferred tail ops (from previous step) per engine
                tail_dve = []
                tail_act = []
                tail_pool = []

                for blk in range(nblk):
                    # gi block: [128, blk_sz, MT, BQ] fp16 staged from giD
                    gi_blk = giblk_pool.tile([128, blk_sz, MT, BQ], F16, name="gi_blk", tag="gi_blk")
                    for m in range(MT):
                        nc.sync.dma_start(
                            gi_blk[:, :, m, :],
                            giD[m, :, blk * blk_sz * BQ:(blk + 1) * blk_sz * BQ].rearrange(
                                "p (t b) -> p t b", b=BQ
                            ),
                        )
                    for tl in range(blk_sz):
                        t = blk * blk_sz + tl
                        cur, nxt = t % 2, (t + 1) % 2

                        # full-bank psum tiles so each group's bank is private
                        phs = [gh_pool.tile([128, 512], F32, name="gh", tag="gh") for _ in range(NG)]
                        last_phs = phs

                        # Previous step's deferred chunk-5 chain must be
                        # emitted BEFORE this burst so tile orders its h'
                        # writes ahead of the k=4,5 MM reads.
                        for f in tail_act:
                            f()
                        for f in tail_dve:
                            f()
                        for f in tail_pool:
                            f()
                        tail_act, tail_dve, tail_pool = [], [], []

                        # ---- MM burst within each group ----
                        import os as _os
                        _ko = _os.environ.get("MM_ORDER", "ko") == "ko"
                        for G in range(NG):
                            if _ko:
                                seq = [(k, lm) for k in range(KT) for lm in range(6)]
                            else:
                                seq = [(k, lm) for lm in range(6) for k in range(KT)]
                            for i, (k, lm) in enumerate(seq):
                                m = G * 6 + lm
                                col = lm * BQ
                                nc.tensor.matmul(
                                    phs[G][:, col:col + BQ],
                                    whh_t[:, k * W3D + m * 128:k * W3D + (m + 1) * 128],
                                    h_st[cur][:, k, :],
                                    start=(i == 0),
                                    stop=(i == len(seq) - 1),
                                )

                        # ---- gate chains, pipelined across engines ----
                        # flush deferred tails from previous step first where
                        # their readiness precedes this step's early ops.
                        new_tail_dve, new_tail_act, new_tail_pool = [], [], []

                        def gi_ap(G, lo, hi):
                            return gi_blk[:, tl, G * 6 + lo:G * 6 + hi, :]

                        arz_t, rz_t, u_t, v_t, n_t, q_t, zh_t, qn_t = [], [], [], [], [], [], [], []
                        for G in range(NG):
                            arz_t.append(tmp.tile([128, 2 * CPG, BQ], F32, name=f"arz{G}", tag=f"arz{G}"))
                            rz_t.append(tmp.tile([128, 2 * CPG, BQ], F32, name=f"rz{G}", tag=f"rz{G}"))
                            u_t.append(tmp.tile([128, CPG, BQ], F32, name=f"u{G}", tag=f"u{G}"))
                            v_t.append(tmp.tile([128, CPG, BQ], F32, name=f"v{G}", tag=f"v{G}"))
                            n_t.append(tmp.tile([128, CPG, BQ], F32, name=f"n{G}", tag=f"n{G}"))
                            q_t.append(tmp.tile([128, CPG, BQ], F32, name=f"q{G}", tag=f"q{G}"))
                            zh_t.append(tmp.tile([128, CPG, BQ], F32, name=f"zh{G}", tag=f"zh{G}"))
                            qn_t.append(tmp.tile([128, CPG, BQ], F32, name=f"qn{G}", tag=f"qn{G}"))

                        def ph3(G):
                            # gate view of the first 96 psum cols: [128, 6, BQ]
                            return phs[G][:, 0:6 * BQ].rearrange("p (m b) -> p m b", b=BQ)

                        def z_ap(G):
                            # z block of rz_t[G]: [128, CPG, BQ]
                            return rz_t[G][:, CPG:2 * CPG, :]

                        def emit_arz(G):
                            nc.vector.tensor_add(
                                arz_t[G][:, :, :], ph3(G)[:, 0:2 * CPG, :], gi_ap(G, 0, 2 * CPG))

                        def emit_sig(G):
                            nc.scalar.activation(rz_t[G][:, :, :], arz_t[G][:, :, :], AF.Sigmoid)

                        def emit_u(G):
                            # u = (gh_n + bn) * r; scalar_tensor_tensor needs a
                            # [128,1] per-partition scalar -> one op per chunk.
                            for cl in range(CPG):
                                nc.vector.scalar_tensor_tensor(
                                    u_t[G][:, cl, :],
                                    ph3(G)[:, 2 * CPG + cl, :],
                                    bn_t[:, 2 * G + cl:2 * G + cl + 1],
                                    rz_t[G][:, cl, :],
                                    ALU.add, ALU.mult,
                                )

                        def emit_v(G):
                            nc.gpsimd.tensor_add(
                                v_t[G][:, :, :], u_t[G][:, :, :], gi_ap(G, 2 * CPG, 3 * CPG))

                        def emit_tanh(G):
                            nc.scalar.activation(n_t[G][:, :, :], v_t[G][:, :, :], AF.Tanh)

                        def emit_q(G):
                            nc.gpsimd.tensor_scalar(
                                q_t[G][:, :, :], z_ap(G), -1.0, 1.0, ALU.mult, ALU.add)

                        def emit_zh(G):
                            nc.gpsimd.tensor_mul(
                                zh_t[G][:, :, :], z_ap(G), h_st[cur][:, 2 * G:2 * G + CPG, :])

                        def emit_qn(G):
                            nc.vector.tensor_mul(qn_t[G][:, :, :], q_t[G][:, :, :], n_t[G][:, :, :])

                        def emit_hn(G):
                            nc.vector.tensor_add(
                                h_st[nxt][:, 2 * G:2 * G + CPG, :], qn_t[G][:, :, :], zh_t[G][:, :, :])

                        # ---- engine emission order (pipeline schedule) ----
                        emit_arz(0)
                        emit_sig(0)
                        emit_u(0)
                        emit_v(0)
                        emit_q(0)
                        emit_zh(0)
                        emit_arz(1)
                        emit_sig(1)
                        emit_tanh(0)
                        emit_qn(0)
                        emit_hn(0)
                        emit_u(1)
                        emit_v(1)
                        emit_q(1)
                        emit_zh(1)
                        emit_arz(2)
                        emit_sig(2)
                        emit_tanh(1)
                        emit_qn(1)
                        emit_hn(1)
                        emit_u(2)
                        emit_v(2)
                        emit_q(2)
                        emit_zh(2)
                        import os as _os4
                        if t == steps - 1 or _os4.environ.get("NODEFER", "") == "1":
                            emit_tanh(2)
                            emit_qn(2)
                            emit_hn(2)
                            tail_dve, tail_act, tail_pool = [], [], []
                        else:
                            # bind this step's tiles explicitly: the emit_*
                            # closures read loop variables that are rebound
                            # next iteration.
                            tail_act = [
                                lambda v2=v_t[2], n2=n_t[2]:
                                    nc.scalar.activation(n2[:, :, :], v2[:, :, :], AF.Tanh)
                            ]
                            tail_dve = [
                                lambda q2=q_t[2], n2=n_t[2], qn2=qn_t[2]:
                                    nc.vector.tensor_mul(qn2[:, :, :], q2[:, :, :], n2[:, :, :]),
                                lambda hd=h_st[nxt], qn2=qn_t[2], zh2=zh_t[2]:
                                    nc.vector.tensor_add(hd[:, 4:6, :], qn2[:, :, :], zh2[:, :, :]),
                            ]
                            tail_pool = []

                # final h -> fp32 out
                import os as _os2
                hout = h_pool.tile([128, KT * BQ], F32, name="hout", tag="hout")
                dbg = _os2.environ.get("DEBUG_GH", "")
                if dbg:
                    nc.vector.tensor_copy(hout[:], last_phs[int(dbg)][:, 0:96])
                else:
                    nc.vector.tensor_copy(
                        hout[:].rearrange("p (k b) -> p k b", b=BQ),
                        h_st[steps % 2][:, :, :])
                nc.sync.dma_start(t_out.ap(), hout[:])

    nc.compile()
    return nc


def _pack_encoder(emb, Wih, Whh, bih, bhh):
    """Host-side prep of one encoder's parameters into device layouts."""
    emb_f16 = np.ascontiguousarray(emb.astype(FP16))
    Wp = Wih[_PERM]  # [2304, 768]
    wihT = np.ascontiguousarray(
        Wp.reshape(MT * 128, KT, 128).transpose(2, 1, 0).reshape(128, KT * 3 * D).astype(FP16)
    )
    Wp = Whh[_PERM]
    whhT = np.ascontiguousarray(
        Wp.reshape(MT * 128, KT, 128).transpose(2, 1, 0).reshape(128, KT * 3 * D).astype(FP16)
    )
    # input-phase bias: bih + bhh for r,z rows; bih only for n rows
    bias_vec = (bih + np.concatenate([bhh[:D], bhh[D:2 * D], np.zeros(D, np.float32)]))[_PERM]
    bias_i = np.ascontiguousarray(bias_vec.reshape(MT, 128).T.astype(np.float32))
    bhh_n = bhh[2 * D:]
    bhhn = np.ascontiguousarray(bhh_n.reshape(KT, 128).T.astype(np.float32))
    return emb_f16, wihT, whhT, bias_i, bhhn


_CACHE = {}


def run_device(inputs, steps=S, trace=False):
    """Run the 8-core device program; returns (h_ctx [64,768], h_tgt [64,768], perf)."""
    key = steps
    if key not in _CACHE:
        _CACHE[key] = _build_program(steps)
    nc = _CACHE[key]

    ctx_tok = np.asarray(inputs["ctx"]).astype(np.int16)      # [64, 256]
    tgt_tok = np.asarray(inputs["tgt_seq"]).astype(np.int16)  # [64, 256]

    enc_ctx = _pack_encoder(
        np.asarray(inputs["emb"], np.float32), np.asarray(inputs["Wih"], np.float32),
        np.asarray(inputs["Whh"], np.float32), np.asarray(inputs["bih"], np.float32),
        np.asarray(inputs["bhh"], np.float32),
    )
    enc_tgt = _pack_encoder(
        np.asarray(inputs["t_emb"], np.float32), np.asarray(inputs["t_Wih"], np.float32),
        np.asarray(inputs["t_Whh"], np.float32), np.asarray(inputs["t_bih"], np.float32),
        np.asarray(inputs["t_bhh"], np.float32),
    )

    in_maps = []
    for core in range(N_CORES):
        e, q = core // 4, core % 4
        emb_f16, wihT, whhT, bias_i, bhhn = enc_ctx if e == 0 else enc_tgt
        toks = (ctx_tok if e == 0 else tgt_tok)[q * BQ:(q + 1) * BQ, :]  # [16, 256]
        # gather position i = t*16+b reads idx[i%16, i//16] = toks[b, t]; the
        # [16, NT/16] block must be replicated into each gpsimd core's stripe.
        idx = np.tile(toks, (8, 1))
        in_maps.append({
            "idx": idx,
            "emb": emb_f16,
            "wihT": wihT,
            "whhT": whhT,
            "bias_i": bias_i,
            "bhhn": bhhn,
        })

    res = run_bass_kernel_spmd(nc, in_maps, core_ids=list(range(N_CORES)), trace=trace)

    def unpack_h(outs):
        # out [128, KT*BQ]: out[p, k*BQ + b] = h[b, k*128 + p]
        h = np.zeros((4 * BQ, D), np.float32)
        for q in range(4):
            o = outs[q]["h_out"].reshape(128, KT, BQ)
            h[q * BQ:(q + 1) * BQ, :] = o.transpose(2, 1, 0).reshape(BQ, D)
        return h

    h_ctx = unpack_h(res.results[0:4])
    h_tgt = unpack_h(res.results[4:8])
    return h_ctx, h_tgt, res


def _head(h_ctx, h_tgt, inputs):
    """Final tiny math on host, float64 for exactness."""
    Wfc = np.asarray(inputs["Wfc"], np.float64)
    bfc = np.asarray(inputs["bfc"], np.float64)
    tWfc = np.asarray(inputs["t_Wfc"], np.float64)
    tbfc = np.asarray(inputs["t_bfc"], np.float64)
    We = np.asarray(inputs["We"], np.float64)
    u0 = np.asarray(inputs["u_sn"], np.float64)

    ctx_latent = h_ctx.astype(np.float64) @ Wfc.T + bfc          # [64, 8]
    target_latent = h_tgt.astype(np.float64) @ tWfc.T + tbfc     # [64, 8]

    u = u0 / (np.linalg.norm(u0) + 1e-12)
    for _ in range(PI):
        v = We.T @ u
        v = v / (np.linalg.norm(v) + 1e-12)
        u = We @ v
        u = u / (np.linalg.norm(u) + 1e-12)
    sigma = u @ (We @ v)
    Wsn = We / sigma

    pred_latent = -(STEPS_DESC * DT_STEP) * (ctx_latent @ Wsn.T)  # [64, 8]
    return (
        pred_latent.astype(np.float32)[:, None, :],
        target_latent.astype(np.float32)[:, None, :],
    )


def kernel(**inputs):
    h_ctx, h_tgt, _ = run_device(inputs, steps=S, trace=False)
    return _head(h_ctx, h_tgt, inputs)


# revision 23
# speedup vs baseline: 1.0471x; 1.0471x over previous
"""Trainium2 Bass kernel for nn_CliffordJEPAModel.

Model = two GRU encoders (ctx / tgt) + tiny closed-form head.

Sharding: 8 cores = 2 encoders x 4 batch-quarters (B_local=16), no
cross-core communication.

Per-core program (input GEMM interleaved into the recurrence):
  The 256-step GRU recurrence is the critical path: per step, 108
  LDW+MM pairs (~27ns each warm) + a gate chain whose tail (last
  group's sigmoid/tanh/combine) cannot hide under the burst.  The
  input matmul gi = Wih' @ X^T (864 N=512 MMs, ~190us) is emitted in
  per-step quanta that fill exactly that PE stall, so it costs ~no
  wall-clock.  gi is staged through a block-contiguous DRAM scratch
  (one 9KB/partition DMA per 16-step block).

  Gate math per step, in NG=3 groups of 2 dim-chunks (each group has a
  private PSUM bank so its chain can start as soon as its 36 MMs are
  done): h' = (1-z)*n + z*h with
    u = (gh_n + bhh_n) * r       (fused scalar_tensor_tensor)
    v = u + gi_n ; n = tanh(v)
    q = 1 - z ; zh = z*h ; h' = q*n + zh    (h kept in fp16)

Host does the final tiny head math in numpy (fc -> spectral norm ->
closed-form descent).
"""

import os
import sys

for _p in ("/opt/trn_rl_repo/concourse", "/opt/trn_rl_repo"):
    if _p not in sys.path:
        sys.path.insert(0, _p)

import numpy as np

import concourse.bacc as bacc
import concourse.mybir as mybir
import concourse.tile as tile
from concourse.bass_utils import run_bass_kernel_spmd

FP16 = np.float16

V, D, NB = 32000, 768, 8
B, S = 64, 256
DT_STEP, STEPS_DESC, PI = 0.1, 5, 3

N_CORES = 8
BQ = B // 4          # batch rows per core (16)
KT = D // 128        # 6 k-chunks
NG = 3               # gate groups per step
CPG = KT // NG       # dim-chunks per group (2)
MT = 3 * KT          # 18 m-tiles of 128 gate rows
NT = BQ * S          # tokens per core (4096)
CHT = 512            # tokens per gather/input-matmul chunk
SPC = CHT // BQ      # steps per chunk (32)
BLK = 16             # recurrence steps per gi prefetch block

F32 = mybir.dt.float32
F16 = mybir.dt.float16
I16 = mybir.dt.int16
AF = mybir.ActivationFunctionType
ALU = mybir.AluOpType

# m-tile order: group G covers dim-chunks (2G, 2G+1); within a group the
# m order is [r_c0, r_c1, z_c0, z_c1, n_c0, n_c1]. m_list[m] = (c, g).
m_list = []
for G in range(NG):
    for g in range(3):
        for cl in range(CPG):
            m_list.append((2 * G + cl, g))

_PERM = np.concatenate(
    [np.arange(g * D + c * 128, g * D + (c + 1) * 128) for (c, g) in m_list]
)


def _build_program(steps=S):
    nc = bacc.Bacc("TRN2", target_bir_lowering=False, debug=False, num_devices=N_CORES)

    t_idx = nc.dram_tensor("idx", [128, NT // 16], I16, kind="ExternalInput")
    t_emb = nc.dram_tensor("emb", [V, D], F16, kind="ExternalInput")
    t_wih = nc.dram_tensor("wihT", [128, KT * 3 * D], F16, kind="ExternalInput")
    t_whh = nc.dram_tensor("whhT", [128, KT * 3 * D], F16, kind="ExternalInput")
    t_bi = nc.dram_tensor("bias_i", [128, MT], F32, kind="ExternalInput")
    t_bn = nc.dram_tensor("bhhn", [128, KT], F32, kind="ExternalInput")
    t_out = nc.dram_tensor("h_out", [128, KT * BQ], F32, kind="ExternalOutput")

    W3D = 3 * D  # 2304

    blk_sz = min(BLK, steps)
    nblk = steps // blk_sz
    n_chunks = (steps * BQ + CHT - 1) // CHT          # input chunks needed
    n_pro = min(2, n_chunks)                          # prologue chunks

    with tile.TileContext(nc) as tc:
        with (
            tc.tile_pool(name="const", bufs=1) as const_pool,
            tc.tile_pool(name="dram", bufs=1, space="DRAM") as dram_pool,
            tc.tile_pool(name="xt", bufs=3) as xt_pool,
            tc.tile_pool(name="psum_in", bufs=2, space="PSUM") as psum_in,
            tc.tile_pool(name="gis", bufs=3) as gis_pool,
            tc.tile_pool(name="gh", bufs=2 * NG, space="PSUM") as gh_pool,
            tc.tile_pool(name="giblk", bufs=3) as giblk_pool,
            tc.tile_pool(name="hstate", bufs=1) as h_pool,
            tc.tile_pool(name="tmp", bufs=3) as tmp,
        ):
            idx_t = const_pool.tile([128, NT // 16], I16)
            wih_t = const_pool.tile([128, KT * W3D], F16)
            whh_t = const_pool.tile([128, KT * W3D], F16)
            bi_t = const_pool.tile([128, MT], F32)
            bn_t = const_pool.tile([128, KT], F32)
            nc.sync.dma_start(idx_t[:], t_idx.ap())
            nc.sync.dma_start(wih_t[:], t_wih.ap())
            nc.sync.dma_start(whh_t[:], t_whh.ap())
            nc.sync.dma_start(bi_t[:], t_bi.ap())
            nc.sync.dma_start(bn_t[:], t_bn.ap())

            # gi scratch, block-contiguous: [128][block][m][t_in_blk*16+b]
            giD = dram_pool.tile([128, nblk, MT, blk_sz * BQ], F16)

            # ---------------- input-pipeline emitters ----------------
            xt_tiles = {}

            def emit_gather(c):
                xt = xt_pool.tile([128, KT, CHT], F16, name="xt", tag="xt")
                xt_tiles[c] = xt
                nc.gpsimd.dma_gather(
                    xt[:, :, :],
                    t_emb.ap(),
                    idx_t[:, c * (CHT // 16):(c + 1) * (CHT // 16)],
                    num_idxs=CHT,
                    num_idxs_reg=CHT,
                    elem_size=D,
                    transpose=True,
                )

            ps_tiles = {}

            def emit_in_mm(c, m, k):
                if (c, m) not in ps_tiles:
                    ps_tiles[(c, m)] = psum_in.tile([128, CHT], F32, name="psin", tag="psin")
                nc.tensor.matmul(
                    ps_tiles[(c, m)][:],
                    wih_t[:, k * W3D + m * 128:k * W3D + (m + 1) * 128],
                    xt_tiles[c][:, k, :],
                    start=(k == 0),
                    stop=(k == KT - 1),
                )

            giD_writers = {}

            def emit_in_act_dma(c, m):
                ps = ps_tiles.pop((c, m))
                gs = gis_pool.tile([128, CHT], F16, name="gs", tag="gs")
                nc.scalar.activation(gs[:], ps[:], AF.Identity, bias=bi_t[:, m:m + 1], scale=1.0)
                nb_in_c = CHT // (blk_sz * BQ)
                j_eff = min(nb_in_c, nblk - nb_in_c * c)
                dst = giD[:, nb_in_c * c:nb_in_c * c + j_eff, m, :]
                w = nc.sync.dma_start(
                    dst,
                    gs[:, 0:j_eff * blk_sz * BQ].rearrange("p (j t) -> p j t", j=j_eff))
                for j in range(j_eff):
                    giD_writers.setdefault(nb_in_c * c + j, []).append(w)

            def emit_chunk_full(c):
                emit_gather(c)
                for m in range(MT):
                    for k in range(KT):
                        emit_in_mm(c, m, k)
                    emit_in_act_dma(c, m)

            # per-step input work schedule (chunks n_pro..n_chunks-1 spread
            # over steps, each finishing one 32-step window before use)
            pe_q = [[] for _ in range(max(steps, 1))]     # (c, m, k) matmuls
            act_q = [[] for _ in range(max(steps, 1))]    # (c, m) act+dma
            TOTAL_MM = MT * KT
            SPCe = SPC - 1   # finish a chunk one step before its first
            for c in range(n_pro, n_chunks):   # consumer block-DMA is emitted
                w0 = SPC * (c - n_pro)
                done_m = 0
                for s in range(SPCe):
                    hi = (TOTAL_MM * (s + 1)) // SPCe
                    for i in range((TOTAL_MM * s) // SPCe, hi):
                        m, k = divmod(i, KT)
                        pe_q[w0 + s].append((c, m, k))
                    while (done_m + 1) * KT <= hi:
                        act_q[min(w0 + s + 1, steps - 1)].append((c, done_m))
                        done_m += 1

            # ---------------- prologue ----------------
            for c in range(n_pro):
                emit_chunk_full(c)
            if n_chunks > n_pro:
                emit_gather(n_pro)   # first interleaved chunk's gather

            h_st = [h_pool.tile([128, KT, BQ], F16, name=f"h{i}", tag=f"h{i}") for i in range(2)]
            nc.vector.memset(h_st[0][:], float(os.environ.get("H0CONST", "0.0")))

            gi_tiles = {}

            def emit_giblk(b):
                gi = giblk_pool.tile([128, MT, blk_sz * BQ], F16, name="gi_blk", tag="gi_blk")
                gi_tiles[b] = gi
                r = nc.sync.dma_start(gi[:, :, :], giD[:, b, :, :])
                for w in giD_writers.get(b, []):
                    tile.add_dep_helper(r.ins, w.ins, sync=True, reason="giD block ready")

            emit_giblk(0)
            if nblk > 1:
                emit_giblk(1)

            # deferred tail ops (from previous step) per engine
            tail_dve = []
            tail_act = []

            last_phs = None
            for blk in range(nblk):
                if blk + 2 < nblk:
                    emit_giblk(blk + 2)
                gi_blk = gi_tiles.pop(blk)
                for tl in range(blk_sz):
                    t = blk * blk_sz + tl
                    cur, nxt = t % 2, (t + 1) % 2

                    phs = [gh_pool.tile([128, 512], F32, name="gh", tag="gh") for _ in range(NG)]
                    last_phs = phs

                    # gathers for future chunks, at window starts
                    if t % SPC == 0:
                        c_next = n_pro + t // SPC + 1
                        if c_next < n_chunks:
                            emit_gather(c_next)

                    # deferred input act+dma first on the ACT stream (runs
                    # during the burst while ACT is idle)
                    for (c, m) in act_q[t]:
                        emit_in_act_dma(c, m)

                    # previous step's deferred G2 chain must be emitted
                    # BEFORE the burst (program order defines tile deps:
                    # the burst's k=4,5 MMs read h chunks hn2' writes).
                    for f in tail_act:
                        f()
                    for f in tail_dve:
                        f()
                    tail_dve = []

                    # ---- MM burst: groups sequential, k ascending ----
                    for G in range(NG):
                        for k in range(KT):
                            for lm in range(6):
                                m = G * 6 + lm
                                col = lm * BQ
                                nc.tensor.matmul(
                                    phs[G][:, col:col + BQ],
                                    whh_t[:, k * W3D + m * 128:k * W3D + (m + 1) * 128],
                                    h_st[cur][:, k, :],
                                    start=(k == 0 and lm == 0),
                                    stop=(k == KT - 1 and lm == 5),
                                )
                    # input-MM quantum fills the PE stall after the burst
                    for (c, m, k) in pe_q[t]:
                        emit_in_mm(c, m, k)

                    # ---- gate chains ----
                    def ph3(G):
                        return phs[G][:, 0:6 * BQ].rearrange("p (m b) -> p m b", b=BQ)

                    def gi_ap(G, lo, hi):
                        return gi_blk[:, G * 6 + lo:G * 6 + hi, tl * BQ:(tl + 1) * BQ]

                    arz_t, rz_t, u_t, v_t, n_t, q_t, zh_t, qn_t = [], [], [], [], [], [], [], []
                    for G in range(NG):
                        arz_t.append(tmp.tile([128, 2 * CPG, BQ], F32, name=f"arz{G}", tag=f"arz{G}"))
                        rz_t.append(tmp.tile([128, 2 * CPG, BQ], F32, name=f"rz{G}", tag=f"rz{G}"))
                        u_t.append(tmp.tile([128, CPG, BQ], F32, name=f"u{G}", tag=f"u{G}"))
                        v_t.append(tmp.tile([128, CPG, BQ], F32, name=f"v{G}", tag=f"v{G}"))
                        n_t.append(tmp.tile([128, CPG, BQ], F32, name=f"n{G}", tag=f"n{G}"))
                        q_t.append(tmp.tile([128, CPG, BQ], F32, name=f"q{G}", tag=f"q{G}"))
                        zh_t.append(tmp.tile([128, CPG, BQ], F32, name=f"zh{G}", tag=f"zh{G}"))
                        qn_t.append(tmp.tile([128, CPG, BQ], F32, name=f"qn{G}", tag=f"qn{G}"))

                    def z_ap(G):
                        return rz_t[G][:, CPG:2 * CPG, :]

                    def emit_arz(G):
                        nc.vector.tensor_add(
                            arz_t[G][:, :, :], ph3(G)[:, 0:2 * CPG, :], gi_ap(G, 0, 2 * CPG))

                    def emit_sig(G):
                        nc.scalar.activation(rz_t[G][:, :, :], arz_t[G][:, :, :], AF.Sigmoid)

                    def emit_u(G):
                        for cl in range(CPG):
                            nc.vector.scalar_tensor_tensor(
                                u_t[G][:, cl, :],
                                ph3(G)[:, 2 * CPG + cl, :],
                                bn_t[:, 2 * G + cl:2 * G + cl + 1],
                                rz_t[G][:, cl, :],
                                ALU.add, ALU.mult,
                            )

                    def emit_v(G):
                        nc.gpsimd.tensor_add(
                            v_t[G][:, :, :], u_t[G][:, :, :], gi_ap(G, 2 * CPG, 3 * CPG))

                    def emit_tanh(G):
                        nc.scalar.activation(n_t[G][:, :, :], v_t[G][:, :, :], AF.Tanh)

                    def emit_q(G):
                        nc.gpsimd.tensor_scalar(
                            q_t[G][:, :, :], z_ap(G), -1.0, 1.0, ALU.mult, ALU.add)

                    def emit_zh(G):
                        nc.gpsimd.tensor_mul(
                            zh_t[G][:, :, :], z_ap(G), h_st[cur][:, 2 * G:2 * G + CPG, :])

                    def emit_qn(G):
                        nc.vector.tensor_mul(qn_t[G][:, :, :], q_t[G][:, :, :], n_t[G][:, :, :])

                    def emit_hn(G):
                        nc.vector.tensor_add(
                            h_st[nxt][:, 2 * G:2 * G + CPG, :], qn_t[G][:, :, :], zh_t[G][:, :, :])

                    # engine emission order (per-engine FIFO schedule)
                    emit_arz(0)
                    emit_sig(0)
                    emit_u(0)
                    emit_v(0)
                    emit_q(0)
                    emit_zh(0)
                    emit_arz(1)
                    emit_tanh(0)
                    emit_sig(1)
                    emit_qn(0)
                    emit_hn(0)
                    emit_u(1)
                    emit_v(1)
                    emit_q(1)
                    emit_zh(1)
                    emit_arz(2)
                    emit_tanh(1)
                    emit_sig(2)
                    emit_qn(1)
                    emit_hn(1)
                    emit_u(2)
                    emit_v(2)
                    emit_q(2)
                    emit_zh(2)
                    if t == steps - 1:
                        emit_tanh(2)
                        emit_qn(2)
                        emit_hn(2)
                        tail_dve, tail_act = [], []
                    else:
                        tail_act = [
                            lambda v2=v_t[2], n2=n_t[2]:
                                nc.scalar.activation(n2[:, :, :], v2[:, :, :], AF.Tanh)
                        ]
                        tail_dve = [
                            lambda q2=q_t[2], n2=n_t[2], qn2=qn_t[2]:
                                nc.vector.tensor_mul(qn2[:, :, :], q2[:, :, :], n2[:, :, :]),
                            lambda hd=h_st[nxt], qn2=qn_t[2], zh2=zh_t[2]:
                                nc.vector.tensor_add(hd[:, 4:6, :], qn2[:, :, :], zh2[:, :, :]),
                        ]

            # final h -> fp32 out
            hout = h_pool.tile([128, KT * BQ], F32, name="hout", tag="hout")
            dbg = os.environ.get("DEBUG_GH", "")
            if dbg:
                nc.vector.tensor_copy(hout[:], last_phs[int(dbg)][:, 0:96])
            else:
                nc.vector.tensor_copy(
                    hout[:].rearrange("p (k b) -> p k b", b=BQ),
                    h_st[steps % 2][:, :, :])
            nc.sync.dma_start(t_out.ap(), hout[:])

    nc.compile()
    return nc


def _pack_encoder(emb, Wih, Whh, bih, bhh):
    """Host-side prep of one encoder's parameters into device layouts."""
    emb_f16 = np.ascontiguousarray(emb.astype(FP16))
    Wp = Wih[_PERM]  # [2304, 768]
    wihT = np.ascontiguousarray(
        Wp.reshape(MT * 128, KT, 128).transpose(2, 1, 0).reshape(128, KT * 3 * D).astype(FP16)
    )
    Wp = Whh[_PERM]
    whhT = np.ascontiguousarray(
        Wp.reshape(MT * 128, KT, 128).transpose(2, 1, 0).reshape(128, KT * 3 * D).astype(FP16)
    )
    # input-phase bias: bih + bhh for r,z rows; bih only for n rows
    bias_vec = (bih + np.concatenate([bhh[:D], bhh[D:2 * D], np.zeros(D, np.float32)]))[_PERM]
    bias_i = np.ascontiguousarray(bias_vec.reshape(MT, 128).T.astype(np.float32))
    bhh_n = bhh[2 * D:]
    bhhn = np.ascontiguousarray(bhh_n.reshape(KT, 128).T.astype(np.float32))
    return emb_f16, wihT, whhT, bias_i, bhhn


_CACHE = {}


def run_device(inputs, steps=S, trace=False):
    """Run the 8-core device program; returns (h_ctx [64,768], h_tgt [64,768], perf)."""
    key = steps
    if key not in _CACHE:
        _CACHE[key] = _build_program(steps)
    nc = _CACHE[key]

    ctx_tok = np.asarray(inputs["ctx"]).astype(np.int16)      # [64, 256]
    tgt_tok = np.asarray(inputs["tgt_seq"]).astype(np.int16)  # [64, 256]

    enc_ctx = _pack_encoder(
        np.asarray(inputs["emb"], np.float32), np.asarray(inputs["Wih"], np.float32),
        np.asarray(inputs["Whh"], np.float32), np.asarray(inputs["bih"], np.float32),
        np.asarray(inputs["bhh"], np.float32),
    )
    enc_tgt = _pack_encoder(
        np.asarray(inputs["t_emb"], np.float32), np.asarray(inputs["t_Wih"], np.float32),
        np.asarray(inputs["t_Whh"], np.float32), np.asarray(inputs["t_bih"], np.float32),
        np.asarray(inputs["t_bhh"], np.float32),
    )

    in_maps = []
    for core in range(N_CORES):
        e, q = core // 4, core % 4
        emb_f16, wihT, whhT, bias_i, bhhn = enc_ctx if e == 0 else enc_tgt
        toks = (ctx_tok if e == 0 else tgt_tok)[q * BQ:(q + 1) * BQ, :]  # [16, 256]
        idx = np.tile(toks, (8, 1))
        in_maps.append({
            "idx": idx,
            "emb": emb_f16,
            "wihT": wihT,
            "whhT": whhT,
            "bias_i": bias_i,
            "bhhn": bhhn,
        })

    res = run_bass_kernel_spmd(nc, in_maps, core_ids=list(range(N_CORES)), trace=trace)

    def unpack_h(outs):
        h = np.zeros((4 * BQ, D), np.float32)
        for q in range(4):
            o = outs[q]["h_out"].reshape(128, KT, BQ)
            h[q * BQ:(q + 1) * BQ, :] = o.transpose(2, 1, 0).reshape(BQ, D)
        return h

    h_ctx = unpack_h(res.results[0:4])
    h_tgt = unpack_h(res.results[4:8])
    return h_ctx, h_tgt, res


def _head(h_ctx, h_tgt, inputs):
    """Final tiny math on host, float64 for exactness."""
    Wfc = np.asarray(inputs["Wfc"], np.float64)
    bfc = np.asarray(inputs["bfc"], np.float64)
    tWfc = np.asarray(inputs["t_Wfc"], np.float64)
    tbfc = np.asarray(inputs["t_bfc"], np.float64)
    We = np.asarray(inputs["We"], np.float64)
    u0 = np.asarray(inputs["u_sn"], np.float64)

    ctx_latent = h_ctx.astype(np.float64) @ Wfc.T + bfc          # [64, 8]
    target_latent = h_tgt.astype(np.float64) @ tWfc.T + tbfc     # [64, 8]

    u = u0 / (np.linalg.norm(u0) + 1e-12)
    for _ in range(PI):
        v = We.T @ u
        v = v / (np.linalg.norm(v) + 1e-12)
        u = We @ v
        u = u / (np.linalg.norm(u) + 1e-12)
    sigma = u @ (We @ v)
    Wsn = We / sigma

    pred_latent = -(STEPS_DESC * DT_STEP) * (ctx_latent @ Wsn.T)  # [64, 8]
    return (
        pred_latent.astype(np.float32)[:, None, :],
        target_latent.astype(np.float32)[:, None, :],
    )


def kernel(**inputs):
    h_ctx, h_tgt, _ = run_device(inputs, steps=S, trace=False)
    return _head(h_ctx, h_tgt, inputs)
